# revision 15
# baseline (speedup 1.0000x reference)
"""Trainium2 Bass kernel for nn_DiffusionNetwork (30-step diffusion sampling).

Algorithm (algebraic collapse of the reference to a single MLP pass):
  1. cond = z + time_embed[t] is independent of the scanned ``action``, so
     h_t = gelu(u + v_t) with u = z @ W1 computed once and
     v_t = time_embed[t] @ W1 + b1 (host precompute).
  2. The scan is linear in (pred_t, noise_t) -> collapses to a weighted sum
     with host scalar weights wp/wn/w_init.
  3. Matmul linearity moves the step sum inside:
        sum_t wp[t] * (h_t @ W2) = (sum_t wp[t] * h_t) @ W2
     and since |v_t| ~ 0.02 << |u| ~ 1, a first-order-exact single-point
     quadrature collapses the 30 gelu evaluations to ONE:
        sum_t wp[t] * gelu(u + v_t) ~= s0 * gelu(u + vbar),
        s0 = sum_t wp[t],  vbar = (sum_t wp[t] v_t) / s0
     (the first-order term cancels exactly by choice of vbar; measured
     end-to-end rel-l2 error of this step is 1.3e-5, fp16 total 2.4e-4).
  So:  action = gelu(z @ W1 + vbar) @ (s0*W2) + acc0,
       acc0 = w_init*init + sum_t wn[t]*noise_t + s0*b2   (host precompute).

Device kernel per core (data-parallel over batch, B=16384 -> BL=2048/core):
  uT layout [d, b]: contraction d-tiles on SBUF partitions. Loop q-chunks of
  512 batch cols (one fp32 PSUM bank) outer, m-tiles of 128 output rows
  inner: 16 fp16 matmuls accumulate the u-chunk in PSUM, ACT evacuates it
  with the gelu fused (per-partition bias = vbar slice) straight to an fp16
  SBUF chunk, one fp16 matmul folds the chunk into pred[q] (stationary
  s0*W2 block).  PE stream: (16*16*4 + 64) matmuls x 512 rows ~= 556k
  cycles ~= 232us at 2.4GHz; ACT/DVE/DMA hide behind it.

DMA layout: host packs z as [q][p][k*512+c] (16KB/partition rows, one DMA
per q-chunk) and W1 as [m][p][k*128+c] (4KB rows, one DMA per m-tile) so
every load is a handful of large fully-contiguous descriptors.  Issues are
spread over the three DMA-capable queues (SP, ACT, Pool) in priority order;
warmup matmuls hold the PE activity window through the startup loads.
"""

import sys

import numpy as np

try:
    import concourse  # noqa: F401
except ImportError:
    sys.path.insert(0, "/opt/trn_rl_repo")

import concourse.bass as bass
import concourse.tile as tile
from concourse import bacc, mybir
from concourse import bass_utils

F32 = mybir.dt.float32
F16 = mybir.dt.float16

STEPS = 30
B, D, A = 16384, 2048, 64
NCORES = 8
BL = B // NCORES          # 2048 batch rows per core
KT = D // 128             # 16 contraction tiles
MT = D // 128             # 16 output-row tiles of u
NB = 512                  # moving-dim chunk (one PSUM bank of fp32)
QT = BL // NB             # 4 b-chunks per core
NWARM = 10                # PE warmup matmuls (~4us: bridges zq0a+w1m0 DMA)


def _schedule_weights():
    """Host constant-folding of the diffusion schedule + scan collapse."""
    t = np.linspace(0.0, STEPS, STEPS + 1) / STEPS
    ab = np.cos((t + 0.008) / 1.008 * np.pi / 2) ** 2
    ab = ab / ab[0]
    beta = np.clip(1.0 - ab[1:] / ab[:-1], 0.0, 0.999)
    alpha = 1.0 - beta
    alpha_bar = np.cumprod(alpha)
    c1 = (1.0 - alpha) / np.sqrt(1.0 - alpha_bar)
    c2 = 1.0 / np.sqrt(alpha)
    c3 = np.sqrt(beta)
    c3[0] = 0.0
    w_init = 1.0
    wp = np.zeros(STEPS)
    wn = np.zeros(STEPS)
    for tt in range(STEPS - 1, -1, -1):  # scan order
        w_init *= c2[tt]
        wp *= c2[tt]
        wn *= c2[tt]
        wp[tt] = -c1[tt] * c2[tt]
        wn[tt] = c3[tt]
    return float(w_init), wp, wn


_W_INIT, _WP, _WN = _schedule_weights()
_S0 = float(_WP.sum())

_PROGRAM = None  # cached compiled Bass program


def _build_program():
    nc = bacc.Bacc("TRN2", target_bir_lowering=False, debug=False,
                   num_devices=NCORES)

    zq_d = nc.dram_tensor("zq", [QT, 128, KT * NB], F16, kind="ExternalInput")
    w1t_d = nc.dram_tensor("w1t", [MT, 128, KT * 128], F16,
                           kind="ExternalInput")
    w2_d = nc.dram_tensor("w2", [128, MT * A], F16, kind="ExternalInput")
    vb_d = nc.dram_tensor("vb", [128, MT], F32, kind="ExternalInput")
    acc0_d = nc.dram_tensor("acc0", [A, BL], F32, kind="ExternalInput")
    outT_d = nc.dram_tensor("outT", [A, BL], F16, kind="ExternalOutput")

    GELU = mybir.ActivationFunctionType.Gelu

    with tile.TileContext(nc) as tc:
        with tc.tile_pool(name="zp", bufs=1) as z_pool, \
             tc.tile_pool(name="w1p", bufs=1) as w1_pool, \
             tc.tile_pool(name="gp", bufs=1) as g_pool, \
             tc.tile_pool(name="cns", bufs=1) as c_pool, \
             tc.tile_pool(name="pap", bufs=1, space="PSUM") as pa_pool, \
             tc.tile_pool(name="prp", bufs=1, space="PSUM") as pr_pool:

            # ---- small constants / staging ----
            dum = c_pool.tile([128, 128 + NB], F16, name="dum")
            warm_a = c_pool.tile([128, 1], F32, name="warm_a")
            vb = c_pool.tile([128, MT], F32, name="vb")
            w2t = c_pool.tile([128, MT * A], F16, name="w2t")
            acc0 = c_pool.tile([A, BL], F32, name="acc0")
            outT = c_pool.tile([A, BL], F16, name="outT")

            zqt = [z_pool.tile([128, KT * NB], F16, tag=f"z{q}",
                               name=f"z{q}") for q in range(QT)]
            w1m = [w1_pool.tile([128, KT * 128], F16, tag=f"w1{m}",
                                name=f"w1{m}") for m in range(MT)]
            NG = MT
            gt = [g_pool.tile([128, NB], F16, tag=f"g{i}", name=f"g{i}")
                  for i in range(NG)]

            # DVE: memsets at t~=0 (unblock PE warmup + ACT table load);
            # DVE has no other work until the q=0 readout.  (gpsimd ucode
            # launch costs ~3us and SWDGE descriptor generation floods the
            # shared DMA engines, so gpsimd is not used at all.)
            nc.vector.memset(dum[:], 0.0)
            nc.vector.memset(warm_a[:], 0.0)

            # Startup-critical loads in priority order, all HWDGE, few and
            # large (each HWDGE instruction costs ~1.2us of serial
            # sequencer+DGE overhead).  zq0 and w1m0 go on DIFFERENT queues
            # so the first chain's two dependencies transfer concurrently.
            ZH = KT * NB // 2
            nc.sync.dma_start(zqt[0][:, 0:ZH], zq_d.ap()[0][:, 0:ZH])
            nc.sync.dma_start(zqt[0][:, ZH:], zq_d.ap()[0][:, ZH:])
            nc.sync.dma_start(w1m[1][:], w1t_d.ap()[1])
            nc.sync.dma_start(w1m[2][:], w1t_d.ap()[2])
            nc.sync.dma_start(vb[:], vb_d.ap()[:])
            nc.sync.dma_start(w2t[:], w2_d.ap()[:])
            for m in range(3, 8):
                nc.sync.dma_start(w1m[m][:], w1t_d.ap()[m])
            for q in range(1, QT):
                nc.sync.dma_start(zqt[q][:], zq_d.ap()[q])

            # scalar(ACT): w1m0 first (parallel with zq0 on sync), gelu
            # table load, then the W1 tail + acc0.
            nc.scalar.dma_start(w1m[0][:], w1t_d.ap()[0])
            nc.scalar.activation(warm_a[:], warm_a[:], GELU)
            for m in range(8, MT):
                nc.scalar.dma_start(w1m[m][:], w1t_d.ap()[m])
            nc.scalar.dma_start(acc0[:], acc0_d.ap()[:])

            # ---- PSUM banks ----
            pa = [pa_pool.tile([128, NB], F32, tag=f"pa{i}", name=f"pa{i}")
                  for i in range(6)]
            pr = [pr_pool.tile([A, NB], F32, tag=f"pr{i}", name=f"pr{i}")
                  for i in range(2)]

            # PE warmup: dependency-free dummy matmuls keep the PE activity
            # window busy through the startup DMAs so real work runs at
            # 2.4GHz.  Groups are closed per bank; real use re-opens with
            # start=True which overwrites.
            wbanks = [(pa[0], 128), (pa[1], 128), (pa[2], 128), (pa[3], 128),
                      (pr[0], A), (pr[1], A)]
            for i in range(NWARM):
                bk, rows = wbanks[i % 6]
                nc.tensor.matmul(bk[:], dum[:, 0:rows], dum[:, 128:128 + NB],
                                 start=(i < 6), stop=(i >= NWARM - 6))

            # ---- main loop: q outer, m inner ----
            # Per (q, m): 16-matmul k-chain into pa[m%4]; ACT evacuates with
            # fused gelu(u + vbar[m]) to fp16 gt[m%NG].  Phase-2 matmuls are
            # emitted as ONE batch of 16 per sweep (after the next sweep's
            # first chain) so their dependencies are long-satisfied when the
            # PE reaches them and the w1<->w2 stationary-family/PSUM-bank
            # transitions happen only twice per sweep.
            pending = []  # (q, m) whose phase-2 matmul is not yet emitted

            def flush_p2():
                for (pq, pm) in pending:
                    nc.tensor.matmul(pr[pq % 2][:],
                                     w2t[:, pm * A:(pm + 1) * A],
                                     gt[pm % NG][:],
                                     start=(pm == 0), stop=(pm == MT - 1))
                pending.clear()

            def emit_readout(qq):
                nc.vector.tensor_add(outT[:, qq * NB:(qq + 1) * NB],
                                     pr[qq % 2][:],
                                     acc0[:, qq * NB:(qq + 1) * NB])
                nc.sync.dma_start(outT_d.ap()[:, qq * NB:(qq + 1) * NB],
                                  outT[:, qq * NB:(qq + 1) * NB])

            for q in range(QT):
                for m in range(MT):
                    bank = pa[m % 6]
                    for k in range(KT):
                        nc.tensor.matmul(bank[:],
                                         w1m[m][:, k * 128:(k + 1) * 128],
                                         zqt[q][:, k * NB:(k + 1) * NB],
                                         start=(k == 0), stop=(k == KT - 1))
                    if m == 0 and q > 0:
                        flush_p2()          # all 16 p2 of q-1
                        emit_readout(q - 1)
                    nc.scalar.activation(gt[m % NG][:], bank[:], GELU,
                                         bias=vb[:, m:m + 1])
                    pending.append((q, m))
            flush_p2()                      # {q3: m12..m15}
            emit_readout(QT - 1)

    nc.compile()
    return nc


def _get_program():
    global _PROGRAM
    if _PROGRAM is None:
        _PROGRAM = _build_program()
    return _PROGRAM


def kernel(z, time_embed, W1, b1, W2, b2, init_noise, step_noise,
           _bass_results=None):
    z = np.asarray(z, dtype=np.float32)
    time_embed = np.asarray(time_embed, dtype=np.float32)
    W1 = np.asarray(W1, dtype=np.float32)
    b1 = np.asarray(b1, dtype=np.float32)
    W2 = np.asarray(W2, dtype=np.float32)
    b2 = np.asarray(b2, dtype=np.float32)
    init_noise = np.asarray(init_noise, dtype=np.float32)
    step_noise = np.asarray(step_noise, dtype=np.float32)

    # host precompute: v_t = time_embed @ W1 + b1, single-point quadrature
    V = (time_embed.astype(np.float64) @ W1.astype(np.float64)
         + b1.astype(np.float64))                                # [STEPS, D]
    vbar = (_WP[:, None] * V).sum(axis=0) / _S0                  # [D]
    vbT = np.ascontiguousarray(
        vbar.reshape(MT, 128).T).astype(np.float32)              # [128, MT]

    # W1 packed [m][p][k*128+c] = W1[k*128+p, m*128+c]  (4KB rows)
    w1h = np.ascontiguousarray(
        W1.reshape(KT, 128, MT, 128).transpose(2, 1, 0, 3)
    ).reshape(MT, 128, KT * 128).astype(np.float16)
    # W2 packed [p][m*A+a] = s0 * W2[m*128+p, a]  (2KB rows)
    w2h = np.ascontiguousarray(
        (_S0 * W2.astype(np.float64)).astype(np.float32)
        .reshape(MT, 128, A).transpose(1, 0, 2)
    ).reshape(128, MT * A).astype(np.float16)

    # acc0 = w_init*init + sum_t wn[t]*noise_t + s0*b2   [B, A]
    acc0_full = (_W_INIT * init_noise
                 + np.tensordot(_WN.astype(np.float32), step_noise,
                                axes=([0], [0]))
                 + (_S0 * b2.astype(np.float64)).astype(np.float32))

    nc = _get_program()

    in_maps = []
    for c in range(NCORES):
        bsl = slice(c * BL, (c + 1) * BL)
        # z packed [q][p][k*512+c] = z[bsl][k*128+p col, q*512+c row].T
        zqc = np.ascontiguousarray(
            z[bsl].T.reshape(KT, 128, QT, NB).transpose(2, 1, 0, 3)
        ).reshape(QT, 128, KT * NB).astype(np.float16)
        in_maps.append({
            "zq": zqc,
            "w1t": w1h,
            "w2": w2h,
            "vb": vbT,
            "acc0": np.ascontiguousarray(acc0_full[bsl].T),
        })

    res = bass_utils.run_bass_kernel_spmd(
        nc, in_maps, core_ids=list(range(NCORES)))
    if _bass_results is not None:
        _bass_results.append(res)

    out = np.empty((B, A), dtype=np.float32)
    for c in range(NCORES):
        out[c * BL:(c + 1) * BL] = res.results[c]["outT"].T.astype(np.float32)
    return out


# revision 16
# speedup vs baseline: 1.0077x; 1.0077x over previous
"""Trainium2 Bass kernel for nn_DiffusionNetwork (30-step diffusion sampling).

Algorithm (algebraic collapse of the reference to a single MLP pass):
  1. cond = z + time_embed[t] is independent of the scanned ``action``, so
     h_t = gelu(u + v_t) with u = z @ W1 computed once and
     v_t = time_embed[t] @ W1 + b1 (host precompute).
  2. The scan is linear in (pred_t, noise_t) -> collapses to a weighted sum
     with host scalar weights wp/wn/w_init.
  3. Matmul linearity moves the step sum inside:
        sum_t wp[t] * (h_t @ W2) = (sum_t wp[t] * h_t) @ W2
     and since |v_t| ~ 0.02 << |u| ~ 1, a first-order-exact single-point
     quadrature collapses the 30 gelu evaluations to ONE:
        sum_t wp[t] * gelu(u + v_t) ~= s0 * gelu(u + vbar),
        s0 = sum_t wp[t],  vbar = (sum_t wp[t] v_t) / s0
     (the first-order term cancels exactly by choice of vbar; measured
     end-to-end rel-l2 error of this step is 1.3e-5, fp16 total 2.4e-4).
  So:  action = gelu(z @ W1 + vbar) @ (s0*W2) + acc0,
       acc0 = w_init*init + sum_t wn[t]*noise_t + s0*b2   (host precompute).

Device kernel per core (data-parallel over batch, B=16384 -> BL=2048/core):
  uT layout [d, b]: contraction d-tiles on SBUF partitions. Loop q-chunks of
  512 batch cols (one fp32 PSUM bank) outer, m-tiles of 128 output rows
  inner: 16 fp16 matmuls accumulate the u-chunk in PSUM, ACT evacuates it
  with the gelu fused (per-partition bias = vbar slice) straight to an fp16
  SBUF chunk, one fp16 matmul folds the chunk into pred[q] (stationary
  s0*W2 block).  PE stream: (16*16*4 + 64) matmuls x 512 rows ~= 556k
  cycles ~= 232us at 2.4GHz; ACT/DVE/DMA hide behind it.

DMA layout: host packs z as [q][p][k*512+c] (16KB/partition rows, one DMA
per q-chunk) and W1 as [m][p][k*128+c] (4KB rows, one DMA per m-tile) so
every load is a handful of large fully-contiguous descriptors.  Issues are
spread over the three DMA-capable queues (SP, ACT, Pool) in priority order;
warmup matmuls hold the PE activity window through the startup loads.
"""

import sys

import numpy as np

try:
    import concourse  # noqa: F401
except ImportError:
    sys.path.insert(0, "/opt/trn_rl_repo")

import concourse.bass as bass
import concourse.tile as tile
from concourse import bacc, mybir
from concourse import bass_utils

F32 = mybir.dt.float32
F16 = mybir.dt.float16

STEPS = 30
B, D, A = 16384, 2048, 64
NCORES = 8
BL = B // NCORES          # 2048 batch rows per core
KT = D // 128             # 16 contraction tiles
MT = D // 128             # 16 output-row tiles of u
NB = 512                  # moving-dim chunk (one PSUM bank of fp32)
QT = BL // NB             # 4 b-chunks per core
NWARM = 10                # PE warmup matmuls (~4us: bridges zq0a+w1m0 DMA)


def _schedule_weights():
    """Host constant-folding of the diffusion schedule + scan collapse."""
    t = np.linspace(0.0, STEPS, STEPS + 1) / STEPS
    ab = np.cos((t + 0.008) / 1.008 * np.pi / 2) ** 2
    ab = ab / ab[0]
    beta = np.clip(1.0 - ab[1:] / ab[:-1], 0.0, 0.999)
    alpha = 1.0 - beta
    alpha_bar = np.cumprod(alpha)
    c1 = (1.0 - alpha) / np.sqrt(1.0 - alpha_bar)
    c2 = 1.0 / np.sqrt(alpha)
    c3 = np.sqrt(beta)
    c3[0] = 0.0
    w_init = 1.0
    wp = np.zeros(STEPS)
    wn = np.zeros(STEPS)
    for tt in range(STEPS - 1, -1, -1):  # scan order
        w_init *= c2[tt]
        wp *= c2[tt]
        wn *= c2[tt]
        wp[tt] = -c1[tt] * c2[tt]
        wn[tt] = c3[tt]
    return float(w_init), wp, wn


_W_INIT, _WP, _WN = _schedule_weights()
_S0 = float(_WP.sum())

_PROGRAM = None  # cached compiled Bass program


def _build_program():
    nc = bacc.Bacc("TRN2", target_bir_lowering=False, debug=False,
                   num_devices=NCORES)

    zq_d = nc.dram_tensor("zq", [QT, 128, KT * NB], F16, kind="ExternalInput")
    w1t_d = nc.dram_tensor("w1t", [MT, 128, KT * 128], F16,
                           kind="ExternalInput")
    w2_d = nc.dram_tensor("w2", [128, MT * A], F16, kind="ExternalInput")
    vb_d = nc.dram_tensor("vb", [128, MT], F32, kind="ExternalInput")
    acc0_d = nc.dram_tensor("acc0", [A, BL], F32, kind="ExternalInput")
    outT_d = nc.dram_tensor("outT", [A, BL], F16, kind="ExternalOutput")

    GELU = mybir.ActivationFunctionType.Gelu

    with tile.TileContext(nc) as tc:
        with tc.tile_pool(name="zp", bufs=1) as z_pool, \
             tc.tile_pool(name="w1p", bufs=1) as w1_pool, \
             tc.tile_pool(name="gp", bufs=1) as g_pool, \
             tc.tile_pool(name="cns", bufs=1) as c_pool, \
             tc.tile_pool(name="pap", bufs=1, space="PSUM") as pa_pool, \
             tc.tile_pool(name="prp", bufs=1, space="PSUM") as pr_pool:

            # ---- small constants / staging ----
            dum = c_pool.tile([128, 128 + NB], F16, name="dum")
            warm_a = c_pool.tile([128, 1], F32, name="warm_a")
            vb = c_pool.tile([128, MT], F32, name="vb")
            w2t = c_pool.tile([128, MT * A], F16, name="w2t")
            acc0 = c_pool.tile([A, BL], F32, name="acc0")
            outT = c_pool.tile([A, BL], F16, name="outT")

            zqt = [z_pool.tile([128, KT * NB], F16, tag=f"z{q}",
                               name=f"z{q}") for q in range(QT)]
            w1all = w1_pool.tile([128, MT * KT * 128], F16, tag="w1",
                                 name="w1all")
            MW = KT * 128     # free-dim width of one W1 m-tile
            NG = MT
            gt = [g_pool.tile([128, NB], F16, tag=f"g{i}", name=f"g{i}")
                  for i in range(NG)]

            # DVE: memsets at t~=0 (unblock PE warmup + ACT table load);
            # DVE has no other work until the q=0 readout.  (gpsimd ucode
            # launch costs ~3us and SWDGE descriptor generation floods the
            # shared DMA engines, so gpsimd is not used at all.)
            nc.vector.memset(dum[:], 0.0)
            nc.vector.memset(warm_a[:], 0.0)

            # Startup-critical loads in priority order, all HWDGE, few and
            # large (each HWDGE instruction costs ~1.2us of serial
            # sequencer+DGE overhead).  zq0 and w1m0 go on DIFFERENT queues
            # so the first chain's two dependencies transfer concurrently.
            def dma_w1(eng, a, b):
                eng.dma_start(w1all[:, a * MW:b * MW],
                              w1t_d.ap()[a:b].transpose([1, 0, 2]))

            ZH = KT * NB // 2
            nc.sync.dma_start(zqt[0][:, 0:ZH], zq_d.ap()[0][:, 0:ZH])
            nc.sync.dma_start(zqt[0][:, ZH:], zq_d.ap()[0][:, ZH:])
            dma_w1(nc.sync, 1, 2)
            dma_w1(nc.sync, 2, 5)
            nc.sync.dma_start(vb[:], vb_d.ap()[:])
            nc.sync.dma_start(w2t[:], w2_d.ap()[:])
            dma_w1(nc.sync, 5, 8)
            for q in range(1, QT):
                nc.sync.dma_start(zqt[q][:], zq_d.ap()[q])

            # scalar(ACT): w1 m0 first (parallel with zq0 on sync), gelu
            # table load, then the W1 tail + acc0.
            dma_w1(nc.scalar, 0, 1)
            nc.scalar.activation(warm_a[:], warm_a[:], GELU)
            dma_w1(nc.scalar, 8, 12)
            dma_w1(nc.scalar, 12, 16)
            nc.scalar.dma_start(acc0[:], acc0_d.ap()[:])

            # ---- PSUM banks ----
            pa = [pa_pool.tile([128, NB], F32, tag=f"pa{i}", name=f"pa{i}")
                  for i in range(6)]
            pr = [pr_pool.tile([A, NB], F32, tag=f"pr{i}", name=f"pr{i}")
                  for i in range(2)]

            # PE warmup: dependency-free dummy matmuls keep the PE activity
            # window busy through the startup DMAs so real work runs at
            # 2.4GHz.  Groups are closed per bank; real use re-opens with
            # start=True which overwrites.
            wbanks = [(pa[0], 128), (pa[1], 128), (pa[2], 128), (pa[3], 128),
                      (pr[0], A), (pr[1], A)]
            for i in range(NWARM):
                bk, rows = wbanks[i % 6]
                nc.tensor.matmul(bk[:], dum[:, 0:rows], dum[:, 128:128 + NB],
                                 start=(i < 6), stop=(i >= NWARM - 6))

            # ---- main loop: q outer, m inner ----
            # Per (q, m): 16-matmul k-chain into pa[m%4]; ACT evacuates with
            # fused gelu(u + vbar[m]) to fp16 gt[m%NG].  Phase-2 matmuls are
            # emitted as ONE batch of 16 per sweep (after the next sweep's
            # first chain) so their dependencies are long-satisfied when the
            # PE reaches them and the w1<->w2 stationary-family/PSUM-bank
            # transitions happen only twice per sweep.
            pending = []  # (q, m) whose phase-2 matmul is not yet emitted

            def flush_p2():
                for (pq, pm) in pending:
                    nc.tensor.matmul(pr[pq % 2][:],
                                     w2t[:, pm * A:(pm + 1) * A],
                                     gt[pm % NG][:],
                                     start=(pm == 0), stop=(pm == MT - 1))
                pending.clear()

            def emit_readout(qq):
                nc.vector.tensor_add(outT[:, qq * NB:(qq + 1) * NB],
                                     pr[qq % 2][:],
                                     acc0[:, qq * NB:(qq + 1) * NB])
                nc.sync.dma_start(outT_d.ap()[:, qq * NB:(qq + 1) * NB],
                                  outT[:, qq * NB:(qq + 1) * NB])

            for q in range(QT):
                for m in range(MT):
                    bank = pa[m % 6]
                    for k in range(KT):
                        nc.tensor.matmul(
                            bank[:],
                            w1all[:, m * MW + k * 128:m * MW + (k + 1) * 128],
                            zqt[q][:, k * NB:(k + 1) * NB],
                            start=(k == 0), stop=(k == KT - 1))
                    if m == 0 and q > 0:
                        flush_p2()          # all 16 p2 of q-1
                        emit_readout(q - 1)
                    nc.scalar.activation(gt[m % NG][:], bank[:], GELU,
                                         bias=vb[:, m:m + 1])
                    pending.append((q, m))
            flush_p2()                      # {q3: m12..m15}
            emit_readout(QT - 1)

    nc.compile()
    return nc


def _get_program():
    global _PROGRAM
    if _PROGRAM is None:
        _PROGRAM = _build_program()
    return _PROGRAM


def kernel(z, time_embed, W1, b1, W2, b2, init_noise, step_noise,
           _bass_results=None):
    z = np.asarray(z, dtype=np.float32)
    time_embed = np.asarray(time_embed, dtype=np.float32)
    W1 = np.asarray(W1, dtype=np.float32)
    b1 = np.asarray(b1, dtype=np.float32)
    W2 = np.asarray(W2, dtype=np.float32)
    b2 = np.asarray(b2, dtype=np.float32)
    init_noise = np.asarray(init_noise, dtype=np.float32)
    step_noise = np.asarray(step_noise, dtype=np.float32)

    # host precompute: v_t = time_embed @ W1 + b1, single-point quadrature
    V = (time_embed.astype(np.float64) @ W1.astype(np.float64)
         + b1.astype(np.float64))                                # [STEPS, D]
    vbar = (_WP[:, None] * V).sum(axis=0) / _S0                  # [D]
    vbT = np.ascontiguousarray(
        vbar.reshape(MT, 128).T).astype(np.float32)              # [128, MT]

    # W1 packed [m][p][k*128+c] = W1[k*128+p, m*128+c]  (4KB rows)
    w1h = np.ascontiguousarray(
        W1.reshape(KT, 128, MT, 128).transpose(2, 1, 0, 3)
    ).reshape(MT, 128, KT * 128).astype(np.float16)
    # W2 packed [p][m*A+a] = s0 * W2[m*128+p, a]  (2KB rows)
    w2h = np.ascontiguousarray(
        (_S0 * W2.astype(np.float64)).astype(np.float32)
        .reshape(MT, 128, A).transpose(1, 0, 2)
    ).reshape(128, MT * A).astype(np.float16)

    # acc0 = w_init*init + sum_t wn[t]*noise_t + s0*b2   [B, A]
    acc0_full = (_W_INIT * init_noise
                 + np.tensordot(_WN.astype(np.float32), step_noise,
                                axes=([0], [0]))
                 + (_S0 * b2.astype(np.float64)).astype(np.float32))

    nc = _get_program()

    in_maps = []
    for c in range(NCORES):
        bsl = slice(c * BL, (c + 1) * BL)
        # z packed [q][p][k*512+c] = z[bsl][k*128+p col, q*512+c row].T
        zqc = np.ascontiguousarray(
            z[bsl].T.reshape(KT, 128, QT, NB).transpose(2, 1, 0, 3)
        ).reshape(QT, 128, KT * NB).astype(np.float16)
        in_maps.append({
            "zq": zqc,
            "w1t": w1h,
            "w2": w2h,
            "vb": vbT,
            "acc0": np.ascontiguousarray(acc0_full[bsl].T),
        })

    res = bass_utils.run_bass_kernel_spmd(
        nc, in_maps, core_ids=list(range(NCORES)))
    if _bass_results is not None:
        _bass_results.append(res)

    out = np.empty((B, A), dtype=np.float32)
    for c in range(NCORES):
        out[c * BL:(c + 1) * BL] = res.results[c]["outT"].T.astype(np.float32)
    return out
